# revision 19
# baseline (speedup 1.0000x reference)
"""Trainium2 Bass kernel for nn_MeanAggregator (time-decayed GNN mean aggregation).

Contract: kernel(**inputs) takes the FULL inputs
  nodes [50000] int, neigh_ids [50000,32] int, neigh_times [50000,32] f32,
  features [500000,128] f32
and returns the FULL output [50000,128] f32.

Strategy: data-parallel shard of the batch dim across 8 NeuronCores, feature
table replicated per core. Per 128-row tile, one indirect (gather) DMA pulls
self+neighbor feature rows into SBUF; the time-decay weights are computed in
one batched phase; the weighted sum runs on DVE (scalar_tensor_tensor chain)
with a slice of neighbor slots offloaded to ScalarE(diag build)+TensorE(PSUM
accumulation).
"""
import os
import sys
import types

import numpy as np

# If a caller sets BASS_TRACE without providing antenv.axon_hooks,
# concourse.bass_utils would crash on import; give it a no-op hook module.
try:
    import antenv.axon_hooks  # noqa: F401
except Exception:
    try:
        import antenv
        _mod = types.ModuleType("antenv.axon_hooks")
        _holder = {"v": None}
        _mod.set_axon_ntff_profile_hook = lambda h: _holder.__setitem__("v", h)
        _mod.get_axon_ntff_profile_hook = lambda: _holder["v"]
        sys.modules["antenv.axon_hooks"] = _mod
        antenv.axon_hooks = _mod
    except Exception:
        pass

import concourse.bacc as bacc
import concourse.mybir as mybir
import concourse.tile as tile
from concourse.bass import IndirectOffsetOnAxis
from concourse.bass_utils import run_bass_kernel_spmd
from concourse.mybir import ActivationFunctionType as act
from concourse.mybir import AluOpType as alu

TIME = 100.0
TAU = 100.0
P = 128
F32 = mybir.dt.float32

N_CORES = 8
B = 50000
K = 32
N = 500000
D = 128
B_CORE = B // N_CORES          # 6250
T = -(-B_CORE // P)            # 49 tiles
B_PAD = T * P                  # 6272
ACT_KS = 10                    # neighbor slots routed via ScalarE+TensorE

LAST_RESULT = None


def _build_kernel(tc, outs, ins, n_tiles, act_ks):
    nc = tc.nc
    feats = ins["features"]
    idx = ins["idx"]
    times = ins["times"]
    out = outs["out"]
    KP1 = K + 1
    Tn = n_tiles

    with (
        tc.tile_pool(name="const", bufs=1) as cpool,
        tc.tile_pool(name="gather", bufs=9) as gpool,
        tc.tile_pool(name="wbig", bufs=1) as wpool,
        tc.tile_pool(name="accs", bufs=3) as apool,
        tc.tile_pool(name="diags", bufs=4) as dpool,
        tc.tile_pool(name="outs", bufs=3) as opool,
    ):
        idx_sb = cpool.tile([P, Tn * KP1], mybir.dt.int32, tag="idx")
        nc.sync.dma_start(idx_sb[:], idx[:, :])
        times_sb = cpool.tile([P, Tn * K], F32, tag="times")
        nc.sync.dma_start(times_sb[:], times[:, :])
        neg1 = cpool.tile([P, 1], F32, tag="neg1")
        nc.vector.memset(neg1[:], -1.0)

        # ---- batched weights phase over all tiles ----
        TK = Tn * K
        e_all = wpool.tile([P, TK], F32, tag="e_all")
        nc.scalar.activation(e_all[:], times_sb[:], act.Exp,
                             bias=neg1[:, :], scale=1.0 / TAU)
        mask = wpool.tile([P, TK], F32, tag="mask")
        nc.vector.tensor_scalar(mask[:], times_sb[:], TIME, None, alu.is_le)
        w_all = wpool.tile([P, TK], F32, tag="w_all")
        nc.vector.tensor_tensor(w_all[:], e_all[:], mask[:], alu.mult)

        total = wpool.tile([P, Tn], F32, tag="total")
        nc.vector.tensor_reduce(
            total[:], w_all[:].rearrange("p (t k) -> p t k", k=K),
            axis=mybir.AxisListType.X, op=alu.add)
        iszero = wpool.tile([P, Tn], F32, tag="iszero")
        nc.vector.tensor_scalar(iszero[:], total[:], 0.0, None, alu.is_equal)
        total_adj = wpool.tile([P, Tn], F32, tag="total_adj")
        nc.vector.tensor_tensor(total_adj[:], total[:], iszero[:], alu.add)
        inv_total = wpool.tile([P, Tn], F32, tag="inv_total")
        nc.vector.reciprocal(inv_total[:], total_adj[:])

        wn_all = wpool.tile([P, TK], F32, tag="wn_all")
        nc.vector.tensor_tensor(
            wn_all[:].rearrange("p (t k) -> p t k", k=K),
            w_all[:].rearrange("p (t k) -> p t k", k=K),
            inv_total[:].to_broadcast((P, Tn, K)),
            alu.mult)
        wnsum = wpool.tile([P, Tn], F32, tag="wnsum")
        nc.vector.tensor_reduce(
            wnsum[:], wn_all[:].rearrange("p (t k) -> p t k", k=K),
            axis=mybir.AxisListType.X, op=alu.add)
        denom = wpool.tile([P, Tn], F32, tag="denom")
        nc.vector.tensor_scalar(denom[:], wnsum[:], 1.0, None, alu.add)
        inv_denom = wpool.tile([P, Tn], F32, tag="inv_denom")
        nc.vector.reciprocal(inv_denom[:], denom[:])
        c1 = wpool.tile([P, Tn], F32, tag="c1")
        nc.vector.tensor_tensor(c1[:], inv_total[:], inv_denom[:], alu.mult)

        if act_ks > 0:
            # identity shipped from host: keeps the Pool engine 100% free
            # for SWDGE descriptor generation (the critical path)
            ident = cpool.tile([P, P], F32, tag="ident")
            nc.sync.dma_start(ident[:], ins["ident"][:, :])

        dve_ks = K - act_ks
        with tc.tile_pool(name="psum", bufs=4, space="PSUM") as ppool:
            for t in range(Tn):
                G = gpool.tile([P, KP1 * D], F32, tag="G")
                # HW contract: one offset element consumed per partition per
                # indirect DMA — 128 random rows per instruction max. The
                # ~1.4us/instruction SWDGE descriptor-generation rate on the
                # single Pool queue is the kernel's critical path.
                for j in range(KP1):
                    nc.gpsimd.indirect_dma_start(
                        G[:, j * D:(j + 1) * D],
                        None,
                        feats[:, :],
                        IndirectOffsetOnAxis(
                            ap=idx_sb[:, t * KP1 + j:t * KP1 + j + 1], axis=0),
                    )

                wt = w_all[:, t * K:(t + 1) * K]

                acc = apool.tile([P, D], F32, tag="acc")
                nc.vector.tensor_scalar(
                    acc[:], G[:, D:2 * D], wt[:, 0:1], None, alu.mult)
                for k in range(1, dve_ks):
                    nc.vector.scalar_tensor_tensor(
                        acc[:], G[:, (k + 1) * D:(k + 2) * D], wt[:, k:k + 1],
                        acc[:], op0=alu.mult, op1=alu.add)

                if act_ks > 0:
                    pt = ppool.tile([P, D], F32, tag="pt")
                    for i, k in enumerate(range(dve_ks, K)):
                        diag = dpool.tile([P, P], F32, tag="diag")
                        nc.scalar.activation(diag[:], ident[:], act.Copy,
                                             scale=wt[:, k:k + 1])
                        nc.tensor.matmul(
                            pt[:], diag[:], G[:, (k + 1) * D:(k + 2) * D],
                            start=(i == 0), stop=(i == act_ks - 1))
                    nc.vector.tensor_tensor(acc[:], acc[:], pt[:], alu.add)

                # out = G0 * inv_denom + acc * c1
                accs = apool.tile([P, D], F32, tag="accs")
                nc.vector.tensor_scalar(
                    accs[:], acc[:], c1[:, t:t + 1], None, alu.mult)
                ot = opool.tile([P, D], F32, tag="ot")
                nc.vector.scalar_tensor_tensor(
                    ot[:], G[:, 0:D], inv_denom[:, t:t + 1], accs[:],
                    op0=alu.mult, op1=alu.add)
                nc.sync.dma_start(out[t * P:(t + 1) * P, :], ot[:])


_NC = None


def _get_nc():
    global _NC
    if _NC is None:
        nc = bacc.Bacc("TRN2", target_bir_lowering=False, debug=False,
                       enable_asserts=False)
        feats = nc.dram_tensor("features", [N, D], F32,
                               kind="ExternalInput").ap()
        idx = nc.dram_tensor("idx", [P, T * (K + 1)], mybir.dt.int32,
                             kind="ExternalInput").ap()
        times = nc.dram_tensor("times", [P, T * K], F32,
                               kind="ExternalInput").ap()
        ident = nc.dram_tensor("ident", [P, P], F32,
                               kind="ExternalInput").ap()
        out = nc.dram_tensor("out", [B_PAD, D], F32,
                             kind="ExternalOutput").ap()
        with tile.TileContext(nc) as tc:
            _build_kernel(tc, {"out": out},
                          {"features": feats, "idx": idx, "times": times,
                           "ident": ident},
                          T, ACT_KS)
        nc.compile()
        _NC = nc
    return _NC


def kernel(nodes, neigh_ids, neigh_times, features):
    global LAST_RESULT
    nodes = np.asarray(nodes).astype(np.int32, copy=False)
    neigh_ids = np.asarray(neigh_ids).astype(np.int32, copy=False)
    neigh_times = np.asarray(neigh_times).astype(np.float32, copy=False)
    features = np.ascontiguousarray(np.asarray(features, dtype=np.float32))

    in_maps = []
    for c in range(N_CORES):
        sl = slice(c * B_CORE, (c + 1) * B_CORE)
        idx_all = np.zeros((B_PAD, K + 1), np.int32)
        idx_all[:B_CORE, 0] = nodes[sl]
        idx_all[:B_CORE, 1:] = neigh_ids[sl]
        times_pad = np.full((B_PAD, K), 200.0, np.float32)
        times_pad[:B_CORE] = neigh_times[sl]
        idx_t = np.ascontiguousarray(
            idx_all.reshape(T, P, K + 1).transpose(1, 0, 2).reshape(P, -1))
        times_t = np.ascontiguousarray(
            times_pad.reshape(T, P, K).transpose(1, 0, 2).reshape(P, -1))
        in_maps.append({"features": features, "idx": idx_t, "times": times_t,
                        "ident": np.eye(P, dtype=np.float32)})

    nc = _get_nc()
    res = run_bass_kernel_spmd(nc, in_maps, core_ids=list(range(N_CORES)))
    LAST_RESULT = res
    out = np.concatenate(
        [res.results[c]["out"][:B_CORE] for c in range(N_CORES)], axis=0)
    return out



# revision 20
# speedup vs baseline: 1.0055x; 1.0055x over previous
"""Trainium2 Bass kernel for nn_MeanAggregator (time-decayed GNN mean aggregation).

Contract: kernel(**inputs) takes the FULL inputs
  nodes [50000] int, neigh_ids [50000,32] int, neigh_times [50000,32] f32,
  features [500000,128] f32
and returns the FULL output [50000,128] f32.

Strategy: data-parallel shard of the batch dim across 8 NeuronCores, feature
table replicated per core. Per 128-row tile, one indirect (gather) DMA pulls
self+neighbor feature rows into SBUF; the time-decay weights are computed in
one batched phase; the weighted sum runs on DVE (scalar_tensor_tensor chain)
with a slice of neighbor slots offloaded to ScalarE(diag build)+TensorE(PSUM
accumulation).
"""
import os
import sys
import types

import numpy as np

# If a caller sets BASS_TRACE without providing antenv.axon_hooks,
# concourse.bass_utils would crash on import; give it a no-op hook module.
try:
    import antenv.axon_hooks  # noqa: F401
except Exception:
    try:
        import antenv
        _mod = types.ModuleType("antenv.axon_hooks")
        _holder = {"v": None}
        _mod.set_axon_ntff_profile_hook = lambda h: _holder.__setitem__("v", h)
        _mod.get_axon_ntff_profile_hook = lambda: _holder["v"]
        sys.modules["antenv.axon_hooks"] = _mod
        antenv.axon_hooks = _mod
    except Exception:
        pass

import concourse.bacc as bacc
import concourse.mybir as mybir
import concourse.tile as tile
from concourse.bass import IndirectOffsetOnAxis
from concourse.bass_utils import run_bass_kernel_spmd
from concourse.mybir import ActivationFunctionType as act
from concourse.mybir import AluOpType as alu

TIME = 100.0
TAU = 100.0
P = 128
F32 = mybir.dt.float32

N_CORES = 8
B = 50000
K = 32
N = 500000
D = 128
B_CORE = B // N_CORES          # 6250
T = -(-B_CORE // P)            # 49 tiles
B_PAD = T * P                  # 6272
ACT_KS = 0                     # neighbor slots routed via ScalarE+TensorE

LAST_RESULT = None


def _build_kernel(tc, outs, ins, n_tiles, act_ks):
    nc = tc.nc
    feats = ins["features"]
    idx = ins["idx"]
    times = ins["times"]
    out = outs["out"]
    KP1 = K + 1
    Tn = n_tiles

    with (
        tc.tile_pool(name="const", bufs=1) as cpool,
        tc.tile_pool(name="gather", bufs=9) as gpool,
        tc.tile_pool(name="wbig", bufs=1) as wpool,
        tc.tile_pool(name="accs", bufs=3) as apool,
        tc.tile_pool(name="diags", bufs=4) as dpool,
        tc.tile_pool(name="outs", bufs=3) as opool,
    ):
        idx_sb = cpool.tile([P, Tn * KP1], mybir.dt.int32, tag="idx")
        nc.sync.dma_start(idx_sb[:], idx[:, :])
        times_sb = cpool.tile([P, Tn * K], F32, tag="times")
        nc.sync.dma_start(times_sb[:], times[:, :])
        neg1 = cpool.tile([P, 1], F32, tag="neg1")
        nc.vector.memset(neg1[:], -1.0)

        # ---- batched weights phase over all tiles ----
        TK = Tn * K
        e_all = wpool.tile([P, TK], F32, tag="e_all")
        nc.scalar.activation(e_all[:], times_sb[:], act.Exp,
                             bias=neg1[:, :], scale=1.0 / TAU)
        mask = wpool.tile([P, TK], F32, tag="mask")
        nc.vector.tensor_scalar(mask[:], times_sb[:], TIME, None, alu.is_le)
        w_all = wpool.tile([P, TK], F32, tag="w_all")
        nc.vector.tensor_tensor(w_all[:], e_all[:], mask[:], alu.mult)

        total = wpool.tile([P, Tn], F32, tag="total")
        nc.vector.tensor_reduce(
            total[:], w_all[:].rearrange("p (t k) -> p t k", k=K),
            axis=mybir.AxisListType.X, op=alu.add)
        iszero = wpool.tile([P, Tn], F32, tag="iszero")
        nc.vector.tensor_scalar(iszero[:], total[:], 0.0, None, alu.is_equal)
        total_adj = wpool.tile([P, Tn], F32, tag="total_adj")
        nc.vector.tensor_tensor(total_adj[:], total[:], iszero[:], alu.add)
        inv_total = wpool.tile([P, Tn], F32, tag="inv_total")
        nc.vector.reciprocal(inv_total[:], total_adj[:])

        wn_all = wpool.tile([P, TK], F32, tag="wn_all")
        nc.vector.tensor_tensor(
            wn_all[:].rearrange("p (t k) -> p t k", k=K),
            w_all[:].rearrange("p (t k) -> p t k", k=K),
            inv_total[:].to_broadcast((P, Tn, K)),
            alu.mult)
        wnsum = wpool.tile([P, Tn], F32, tag="wnsum")
        nc.vector.tensor_reduce(
            wnsum[:], wn_all[:].rearrange("p (t k) -> p t k", k=K),
            axis=mybir.AxisListType.X, op=alu.add)
        denom = wpool.tile([P, Tn], F32, tag="denom")
        nc.vector.tensor_scalar(denom[:], wnsum[:], 1.0, None, alu.add)
        inv_denom = wpool.tile([P, Tn], F32, tag="inv_denom")
        nc.vector.reciprocal(inv_denom[:], denom[:])
        c1 = wpool.tile([P, Tn], F32, tag="c1")
        nc.vector.tensor_tensor(c1[:], inv_total[:], inv_denom[:], alu.mult)

        if act_ks > 0:
            # identity shipped from host: keeps the Pool engine 100% free
            # for SWDGE descriptor generation (the critical path)
            ident = cpool.tile([P, P], F32, tag="ident")
            nc.sync.dma_start(ident[:], ins["ident"][:, :])

        dve_ks = K - act_ks
        with tc.tile_pool(name="psum", bufs=4, space="PSUM") as ppool:
            for t in range(Tn):
                G = gpool.tile([P, KP1 * D], F32, tag="G")
                # HW contract: one offset element consumed per partition per
                # indirect DMA — 128 random rows per instruction max. The
                # ~1.4us/instruction SWDGE descriptor-generation rate on the
                # single Pool queue is the kernel's critical path.
                for j in range(KP1):
                    nc.gpsimd.indirect_dma_start(
                        G[:, j * D:(j + 1) * D],
                        None,
                        feats[:, :],
                        IndirectOffsetOnAxis(
                            ap=idx_sb[:, t * KP1 + j:t * KP1 + j + 1], axis=0),
                    )

                wt = w_all[:, t * K:(t + 1) * K]

                acc = apool.tile([P, D], F32, tag="acc")
                nc.vector.tensor_scalar(
                    acc[:], G[:, D:2 * D], wt[:, 0:1], None, alu.mult)
                for k in range(1, dve_ks):
                    nc.vector.scalar_tensor_tensor(
                        acc[:], G[:, (k + 1) * D:(k + 2) * D], wt[:, k:k + 1],
                        acc[:], op0=alu.mult, op1=alu.add)

                if act_ks > 0:
                    pt = ppool.tile([P, D], F32, tag="pt")
                    for i, k in enumerate(range(dve_ks, K)):
                        diag = dpool.tile([P, P], F32, tag="diag")
                        nc.scalar.activation(diag[:], ident[:], act.Copy,
                                             scale=wt[:, k:k + 1])
                        nc.tensor.matmul(
                            pt[:], diag[:], G[:, (k + 1) * D:(k + 2) * D],
                            start=(i == 0), stop=(i == act_ks - 1))
                    nc.vector.tensor_tensor(acc[:], acc[:], pt[:], alu.add)

                # out = G0 * inv_denom + acc * c1
                accs = apool.tile([P, D], F32, tag="accs")
                nc.vector.tensor_scalar(
                    accs[:], acc[:], c1[:, t:t + 1], None, alu.mult)
                ot = opool.tile([P, D], F32, tag="ot")
                nc.vector.scalar_tensor_tensor(
                    ot[:], G[:, 0:D], inv_denom[:, t:t + 1], accs[:],
                    op0=alu.mult, op1=alu.add)
                nc.sync.dma_start(out[t * P:(t + 1) * P, :], ot[:])


_NC = None


def _get_nc():
    global _NC
    if _NC is None:
        nc = bacc.Bacc("TRN2", target_bir_lowering=False, debug=False,
                       enable_asserts=False)
        feats = nc.dram_tensor("features", [N, D], F32,
                               kind="ExternalInput").ap()
        idx = nc.dram_tensor("idx", [P, T * (K + 1)], mybir.dt.int32,
                             kind="ExternalInput").ap()
        times = nc.dram_tensor("times", [P, T * K], F32,
                               kind="ExternalInput").ap()
        ident = nc.dram_tensor("ident", [P, P], F32,
                               kind="ExternalInput").ap()
        out = nc.dram_tensor("out", [B_PAD, D], F32,
                             kind="ExternalOutput").ap()
        with tile.TileContext(nc) as tc:
            _build_kernel(tc, {"out": out},
                          {"features": feats, "idx": idx, "times": times,
                           "ident": ident},
                          T, ACT_KS)
        nc.compile()
        _NC = nc
    return _NC


def kernel(nodes, neigh_ids, neigh_times, features):
    global LAST_RESULT
    nodes = np.asarray(nodes).astype(np.int32, copy=False)
    neigh_ids = np.asarray(neigh_ids).astype(np.int32, copy=False)
    neigh_times = np.asarray(neigh_times).astype(np.float32, copy=False)
    features = np.ascontiguousarray(np.asarray(features, dtype=np.float32))

    in_maps = []
    for c in range(N_CORES):
        sl = slice(c * B_CORE, (c + 1) * B_CORE)
        idx_all = np.zeros((B_PAD, K + 1), np.int32)
        idx_all[:B_CORE, 0] = nodes[sl]
        idx_all[:B_CORE, 1:] = neigh_ids[sl]
        times_pad = np.full((B_PAD, K), 200.0, np.float32)
        times_pad[:B_CORE] = neigh_times[sl]
        idx_t = np.ascontiguousarray(
            idx_all.reshape(T, P, K + 1).transpose(1, 0, 2).reshape(P, -1))
        times_t = np.ascontiguousarray(
            times_pad.reshape(T, P, K).transpose(1, 0, 2).reshape(P, -1))
        in_maps.append({"features": features, "idx": idx_t, "times": times_t,
                        "ident": np.eye(P, dtype=np.float32)})

    nc = _get_nc()
    res = run_bass_kernel_spmd(nc, in_maps, core_ids=list(range(N_CORES)))
    LAST_RESULT = res
    out = np.concatenate(
        [res.results[c]["out"][:B_CORE] for c in range(N_CORES)], axis=0)
    return out



# revision 21
# speedup vs baseline: 1.0241x; 1.0185x over previous
"""Trainium2 Bass kernel for nn_MeanAggregator (time-decayed GNN mean aggregation).

Contract: kernel(**inputs) takes the FULL inputs
  nodes [50000] int, neigh_ids [50000,32] int, neigh_times [50000,32] f32,
  features [500000,128] f32
and returns the FULL output [50000,128] f32.

Strategy: data-parallel shard of the batch dim across 8 NeuronCores, feature
table replicated per core. Per 128-row tile, one indirect (gather) DMA pulls
self+neighbor feature rows into SBUF; the time-decay weights are computed in
one batched phase; the weighted sum runs on DVE (scalar_tensor_tensor chain)
with a slice of neighbor slots offloaded to ScalarE(diag build)+TensorE(PSUM
accumulation).
"""
import os
import sys
import types

import numpy as np

# If a caller sets BASS_TRACE without providing antenv.axon_hooks,
# concourse.bass_utils would crash on import; give it a no-op hook module.
try:
    import antenv.axon_hooks  # noqa: F401
except Exception:
    try:
        import antenv
        _mod = types.ModuleType("antenv.axon_hooks")
        _holder = {"v": None}
        _mod.set_axon_ntff_profile_hook = lambda h: _holder.__setitem__("v", h)
        _mod.get_axon_ntff_profile_hook = lambda: _holder["v"]
        sys.modules["antenv.axon_hooks"] = _mod
        antenv.axon_hooks = _mod
    except Exception:
        pass

import concourse.bacc as bacc
import concourse.mybir as mybir
import concourse.tile as tile
from concourse.bass import IndirectOffsetOnAxis
from concourse.bass_utils import run_bass_kernel_spmd
from concourse.mybir import ActivationFunctionType as act
from concourse.mybir import AluOpType as alu

TIME = 100.0
TAU = 100.0
P = 128
F32 = mybir.dt.float32

N_CORES = 8
B = 50000
K = 32
N = 500000
D = 128
B_CORE = B // N_CORES          # 6250
T = -(-B_CORE // P)            # 49 tiles
B_PAD = T * P                  # 6272
ACT_KS = 10                    # neighbor slots routed via ScalarE+TensorE

LAST_RESULT = None


def _build_kernel(tc, outs, ins, n_tiles, act_ks):
    nc = tc.nc
    feats = ins["features"]
    idx = ins["idx"]
    times = ins["times"]
    out = outs["out"]
    KP1 = K + 1
    Tn = n_tiles

    with (
        tc.tile_pool(name="const", bufs=1) as cpool,
        tc.tile_pool(name="gather", bufs=9) as gpool,
        tc.tile_pool(name="wbig", bufs=1) as wpool,
        tc.tile_pool(name="accs", bufs=3) as apool,
        tc.tile_pool(name="diags", bufs=4) as dpool,
        tc.tile_pool(name="outs", bufs=3) as opool,
    ):
        idx_sb = cpool.tile([P, Tn * KP1], mybir.dt.int32, tag="idx")
        nc.sync.dma_start(idx_sb[:], idx[:, :])
        times_sb = cpool.tile([P, Tn * K], F32, tag="times")
        nc.sync.dma_start(times_sb[:], times[:, :])
        neg1 = cpool.tile([P, 1], F32, tag="neg1")
        nc.vector.memset(neg1[:], -1.0)

        # ---- batched weights phase over all tiles ----
        TK = Tn * K
        e_all = wpool.tile([P, TK], F32, tag="e_all")
        nc.scalar.activation(e_all[:], times_sb[:], act.Exp,
                             bias=neg1[:, :], scale=1.0 / TAU)
        mask = wpool.tile([P, TK], F32, tag="mask")
        nc.vector.tensor_scalar(mask[:], times_sb[:], TIME, None, alu.is_le)
        w_all = wpool.tile([P, TK], F32, tag="w_all")
        nc.vector.tensor_tensor(w_all[:], e_all[:], mask[:], alu.mult)

        total = wpool.tile([P, Tn], F32, tag="total")
        nc.vector.tensor_reduce(
            total[:], w_all[:].rearrange("p (t k) -> p t k", k=K),
            axis=mybir.AxisListType.X, op=alu.add)
        iszero = wpool.tile([P, Tn], F32, tag="iszero")
        nc.vector.tensor_scalar(iszero[:], total[:], 0.0, None, alu.is_equal)
        total_adj = wpool.tile([P, Tn], F32, tag="total_adj")
        nc.vector.tensor_tensor(total_adj[:], total[:], iszero[:], alu.add)
        inv_total = wpool.tile([P, Tn], F32, tag="inv_total")
        nc.vector.reciprocal(inv_total[:], total_adj[:])

        wn_all = wpool.tile([P, TK], F32, tag="wn_all")
        nc.vector.tensor_tensor(
            wn_all[:].rearrange("p (t k) -> p t k", k=K),
            w_all[:].rearrange("p (t k) -> p t k", k=K),
            inv_total[:].to_broadcast((P, Tn, K)),
            alu.mult)
        wnsum = wpool.tile([P, Tn], F32, tag="wnsum")
        nc.vector.tensor_reduce(
            wnsum[:], wn_all[:].rearrange("p (t k) -> p t k", k=K),
            axis=mybir.AxisListType.X, op=alu.add)
        denom = wpool.tile([P, Tn], F32, tag="denom")
        nc.vector.tensor_scalar(denom[:], wnsum[:], 1.0, None, alu.add)
        inv_denom = wpool.tile([P, Tn], F32, tag="inv_denom")
        nc.vector.reciprocal(inv_denom[:], denom[:])
        c1 = wpool.tile([P, Tn], F32, tag="c1")
        nc.vector.tensor_tensor(c1[:], inv_total[:], inv_denom[:], alu.mult)

        if act_ks > 0:
            # identity shipped from host: keeps the Pool engine 100% free
            # for SWDGE descriptor generation (the critical path)
            ident = cpool.tile([P, P], F32, tag="ident")
            nc.sync.dma_start(ident[:], ins["ident"][:, :])

        dve_ks = K - act_ks
        with tc.tile_pool(name="psum", bufs=4, space="PSUM") as ppool:
            for t in range(Tn):
                G = gpool.tile([P, KP1 * D], F32, tag="G")
                # HW contract: one offset element consumed per partition per
                # indirect DMA — 128 random rows per instruction max. The
                # ~1.4us/instruction SWDGE descriptor-generation rate on the
                # single Pool queue is the kernel's critical path.
                for j in range(KP1):
                    nc.gpsimd.indirect_dma_start(
                        G[:, j * D:(j + 1) * D],
                        None,
                        feats[:, :],
                        IndirectOffsetOnAxis(
                            ap=idx_sb[:, t * KP1 + j:t * KP1 + j + 1], axis=0),
                    )

                wt = w_all[:, t * K:(t + 1) * K]

                acc = apool.tile([P, D], F32, tag="acc")
                nc.vector.tensor_scalar(
                    acc[:], G[:, D:2 * D], wt[:, 0:1], None, alu.mult)
                for k in range(1, dve_ks):
                    nc.vector.scalar_tensor_tensor(
                        acc[:], G[:, (k + 1) * D:(k + 2) * D], wt[:, k:k + 1],
                        acc[:], op0=alu.mult, op1=alu.add)

                if act_ks > 0:
                    pt = ppool.tile([P, D], F32, tag="pt")
                    for i, k in enumerate(range(dve_ks, K)):
                        diag = dpool.tile([P, P], F32, tag="diag")
                        nc.scalar.activation(diag[:], ident[:], act.Copy,
                                             scale=wt[:, k:k + 1])
                        nc.tensor.matmul(
                            pt[:], diag[:], G[:, (k + 1) * D:(k + 2) * D],
                            start=(i == 0), stop=(i == act_ks - 1))
                    nc.vector.tensor_tensor(acc[:], acc[:], pt[:], alu.add)

                # out = G0 * inv_denom + acc * c1
                accs = apool.tile([P, D], F32, tag="accs")
                nc.vector.tensor_scalar(
                    accs[:], acc[:], c1[:, t:t + 1], None, alu.mult)
                ot = opool.tile([P, D], F32, tag="ot")
                nc.vector.scalar_tensor_tensor(
                    ot[:], G[:, 0:D], inv_denom[:, t:t + 1], accs[:],
                    op0=alu.mult, op1=alu.add)
                nc.sync.dma_start(out[t * P:(t + 1) * P, :], ot[:])


_NC = None


def _get_nc():
    global _NC
    if _NC is None:
        nc = bacc.Bacc("TRN2", target_bir_lowering=False, debug=False,
                       enable_asserts=False)
        feats = nc.dram_tensor("features", [N, D], F32,
                               kind="ExternalInput").ap()
        idx = nc.dram_tensor("idx", [P, T * (K + 1)], mybir.dt.int32,
                             kind="ExternalInput").ap()
        times = nc.dram_tensor("times", [P, T * K], F32,
                               kind="ExternalInput").ap()
        ident = nc.dram_tensor("ident", [P, P], F32,
                               kind="ExternalInput").ap()
        out = nc.dram_tensor("out", [B_PAD, D], F32,
                             kind="ExternalOutput").ap()
        with tile.TileContext(nc) as tc:
            _build_kernel(tc, {"out": out},
                          {"features": feats, "idx": idx, "times": times,
                           "ident": ident},
                          T, ACT_KS)
        nc.compile()
        _NC = nc
    return _NC


def kernel(nodes, neigh_ids, neigh_times, features):
    global LAST_RESULT
    nodes = np.asarray(nodes).astype(np.int32, copy=False)
    neigh_ids = np.asarray(neigh_ids).astype(np.int32, copy=False)
    neigh_times = np.asarray(neigh_times).astype(np.float32, copy=False)
    features = np.ascontiguousarray(np.asarray(features, dtype=np.float32))

    in_maps = []
    for c in range(N_CORES):
        sl = slice(c * B_CORE, (c + 1) * B_CORE)
        idx_all = np.zeros((B_PAD, K + 1), np.int32)
        idx_all[:B_CORE, 0] = nodes[sl]
        idx_all[:B_CORE, 1:] = neigh_ids[sl]
        times_pad = np.full((B_PAD, K), 200.0, np.float32)
        times_pad[:B_CORE] = neigh_times[sl]
        idx_t = np.ascontiguousarray(
            idx_all.reshape(T, P, K + 1).transpose(1, 0, 2).reshape(P, -1))
        times_t = np.ascontiguousarray(
            times_pad.reshape(T, P, K).transpose(1, 0, 2).reshape(P, -1))
        in_maps.append({"features": features, "idx": idx_t, "times": times_t,
                        "ident": np.eye(P, dtype=np.float32)})

    nc = _get_nc()
    res = run_bass_kernel_spmd(nc, in_maps, core_ids=list(range(N_CORES)))
    LAST_RESULT = res
    out = np.concatenate(
        [res.results[c]["out"][:B_CORE] for c in range(N_CORES)], axis=0)
    return out

